# revision 1
# baseline (speedup 1.0000x reference)
"""Trainium2 Bass kernel for a 2-layer DGL-style GCN (mean aggregation).

Reference computation:
    h_N  = segmean(feat[src] -> dst)                 # [N, 128]
    h    = relu(concat([feat, h_N]) @ W0.T)          # [N, 128]
    h_N2 = segmean(h[src] -> dst)
    out  = concat([h, h_N2]) @ W1.T                  # [N, 64]

Distribution: dst-range sharding over 8 cores (node n owned by core
n // NPC).  Each core aggregates its own nodes exactly; one AllGather
shares h rows (bf16) between the passes.

v2 layout (vs the fp32/GRP=512 baseline): everything bf16, groups of
128 dst nodes (one PSUM quarter-bank per group), so each selection
matmul streams 128 rows at 1 cycle/row instead of 512 at fp32r rates.
Edges are bucketed by (group of 128 dsts, source-table chunk of 25600
rows); per bucket KC slot-blocks of 128.  One dma_gather per
(supergroup of 4 groups, chunk) pulls 4*KC*128 source rows (256B bf16
descriptors); the descriptor ring is enlarged (dynamic_dma_scratch_size)
so 2560-index gathers fit, amortizing the ~1us SWDGE fixed cost.  The
selection matrix M[slot, v] = (dl[slot]==v) * wg[slot] is built by one
DVE tensor_scalar per slot-block ([128,128] bf16, 4x perf mode).
Pass 2 gathers h rows directly (segmean commutes with the linear layer,
so no z2 trick is needed once rows are bf16).  All 8 cores run one
identical program on different data.
"""

import sys

sys.path.insert(0, "/opt/trn_rl_repo")

from contextlib import ExitStack

import numpy as np

import concourse.bass as bass
import concourse.tile as tile
from concourse import bacc, mybir
from concourse.bass_utils import run_bass_kernel_spmd

F32 = mybir.dt.float32
BF16 = mybir.dt.bfloat16
I16 = mybir.dt.int16
P = 128
GRP = 128  # dst nodes per group (one PSUM quarter-bank)
SG = 4  # groups per supergroup (per gather / meta load)


def _np_bf16():
    return mybir.dt.np(BF16)


def _split_sync_waits(nc, max_waits=1):
    """This walrus's codegen rejects instructions carrying more than
    `max_waits` semaphore waits. Hoist the excess onto same-engine nops
    inserted immediately before the offending instruction."""
    import bass_rust

    ctr = 0
    for bb in nc.main_func.blocks:
        insts = bb.instructions
        need = any(
            ins.sync_info is not None and len(ins.sync_info.on_wait) > max_waits
            for ins in insts
        )
        if not need:
            continue
        out = []
        for ins in insts:
            si = ins.sync_info
            if si is not None and len(si.on_wait) > max_waits:
                waits = list(si.on_wait)
                keep, rest = waits[:max_waits], waits[max_waits:]
                while rest:
                    chunk, rest = rest[:max_waits], rest[max_waits:]
                    ctr += 1
                    nop = bass_rust.InstNoOp(
                        name=f"I-waitsplit-{ctr}", engine=ins.engine
                    )
                    nop.sync_info = mybir.SyncInfo(on_wait=chunk, on_update=[])
                    out.append(nop)
                si.on_wait = keep
            out.append(ins)
        insts.clear()
        insts.extend(out)


class _GcnBacc(bacc.Bacc):
    """Bacc whose finalize also splits multi-wait instructions (this
    walrus rejects >1 sync wait on several instruction templates)."""

    def finalize(self):
        if self._finalized:
            return
        self.compile()
        _split_sync_waits(self)
        bass.Bass.finalize(self)


def build_program(cfg, reps=1, no_cc=False):
    """Emit the per-core SPMD program (identical across cores)."""
    N_PAD, D_HID, D_OUT = cfg["N_PAD"], cfg["D_HID"], cfg["D_OUT"]
    C, NPC, KC, NCH = cfg["C"], cfg["NPC"], cfg["KC"], cfg["NCH"]
    CH = N_PAD // NCH  # rows per source-table chunk
    NG = NPC // GRP  # groups per core
    NSG = NG // SG  # supergroups per core
    CAP = KC * P  # edge slots per (group, chunk) bucket
    IW = CAP // 16  # idx16 columns per (group, chunk)
    GIDX = SG * IW  # idx16 cols per (supergroup, chunk)
    NIDX = SG * CAP  # indices per gather
    MC = SG * KC  # dl/wg cols per chunk (per supergroup)

    nc = _GcnBacc(None)
    feat_t = nc.declare_dram_parameter("featbf", [N_PAD, P], BF16, isOutput=False)
    featT_t = nc.declare_dram_parameter("featTbf", [P, NPC], BF16, isOutput=False)
    idx_t = nc.declare_dram_parameter("idx16", [NSG, P, NCH * GIDX], I16, isOutput=False)
    dl_t = nc.declare_dram_parameter("dl", [NSG, P, NCH * MC], F32, isOutput=False)
    wg_t = nc.declare_dram_parameter("wg", [NSG, P, NCH * MC], F32, isOutput=False)
    w0a_t = nc.declare_dram_parameter("w0at", [P, D_HID], BF16, isOutput=False)
    w0b_t = nc.declare_dram_parameter("w0bt", [P, D_HID], BF16, isOutput=False)
    w1a_t = nc.declare_dram_parameter("w1at", [D_HID, D_OUT], BF16, isOutput=False)
    w1b_t = nc.declare_dram_parameter("w1bt", [D_HID, D_OUT], BF16, isOutput=False)
    iota_t = nc.declare_dram_parameter("iota", [P, GRP], BF16, isOutput=False)
    ident_t = nc.declare_dram_parameter("ident", [P, P], BF16, isOutput=False)
    outT_t = nc.declare_dram_parameter("outT", [D_OUT, NPC], F32, isOutput=True)

    with ExitStack() as ctx:
        tc = ctx.enter_context(tile.TileContext(nc))

        const = ctx.enter_context(tc.tile_pool(name="const", bufs=1))
        dram = ctx.enter_context(tc.tile_pool(name="dram", bufs=1, space="DRAM"))
        h_piece = dram.tile([NPC, P], BF16)
        h_full = dram.tile([C * NPC, P], BF16)

        w0a_s = const.tile([P, D_HID], BF16, tag="w0a")
        w0b_s = const.tile([P, D_HID], BF16, tag="w0b")
        w1a_s = const.tile([D_HID, D_OUT], BF16, tag="w1a")
        w1b_s = const.tile([D_HID, D_OUT], BF16, tag="w1b")
        iota_s = const.tile([P, GRP], BF16, tag="iota")
        ident_s = const.tile([P, P], BF16, tag="ident")
        hT_s = const.tile([D_HID, NPC], BF16, tag="hT")
        for dst, src in [
            (w0a_s, w0a_t),
            (w0b_s, w0b_t),
            (w1a_s, w1a_t),
            (w1b_s, w1b_t),
            (iota_s, iota_t),
            (ident_s, ident_t),
        ]:
            nc.sync.dma_start(out=dst[:], in_=src[:])

        idxp = ctx.enter_context(tc.tile_pool(name="idxp", bufs=2))
        ftp = ctx.enter_context(tc.tile_pool(name="ftp", bufs=2))
        gp = ctx.enter_context(tc.tile_pool(name="gp", bufs=2))
        mp = ctx.enter_context(tc.tile_pool(name="mp", bufs=6))
        misc = ctx.enter_context(tc.tile_pool(name="misc", bufs=4))
        pa_p = ctx.enter_context(tc.tile_pool(name="pa", bufs=1, space="PSUM"))
        ph_p = ctx.enter_context(tc.tile_pool(name="ph", bufs=2, space="PSUM"))
        pt_p = ctx.enter_context(tc.tile_pool(name="pt", bufs=1, space="PSUM"))

        qn = [0]

        def agg_supergroup(sg, table_ap, gtag, mtag, patag):
            """Gather + selection-matmul aggregation for one supergroup.
            Returns the 4 per-group PSUM aggregation tiles (stopped)."""
            idx_s = idxp.tile([P, NCH * GIDX], I16, tag=f"idx{gtag}")
            nc.sync.dma_start(out=idx_s[:], in_=idx_t[sg])
            dl_s = idxp.tile([P, NCH * MC], F32, tag=f"dl{gtag}")
            nc.sync.dma_start(out=dl_s[:], in_=dl_t[sg])
            wg_s = idxp.tile([P, NCH * MC], F32, tag=f"wg{gtag}")
            nc.sync.dma_start(out=wg_s[:], in_=wg_t[sg])

            pas = [
                pa_p.tile([P, GRP], F32, tag=f"pa{j}", name=f"{patag}{j}")
                for j in range(SG)
            ]
            for c in range(NCH):
                gt = gp.tile([P, NIDX], BF16, tag=gtag)
                for jg in range(SG):
                    nc.gpsimd.dma_gather(
                        out_ap=gt[:, jg * CAP : (jg + 1) * CAP].rearrange(
                            "p (k e) -> p k e", e=P
                        ),
                        in_ap=table_ap[c * CH : (c + 1) * CH, :],
                        idxs_ap=idx_s[:, c * GIDX + jg * IW : c * GIDX + (jg + 1) * IW],
                        num_idxs=CAP,
                        num_idxs_reg=CAP,
                        elem_size=P,
                    )
                for jg in range(SG):
                    for k in range(KC):
                        col = c * MC + jg * KC + k
                        m = mp.tile([P, GRP], BF16, tag=mtag)
                        nc.vector.tensor_scalar(
                            out=m[:],
                            in0=iota_s[:],
                            scalar1=dl_s[:, col : col + 1],
                            scalar2=wg_s[:, col : col + 1],
                            op0=mybir.AluOpType.is_equal,
                            op1=mybir.AluOpType.mult,
                        )
                        s0 = (jg * KC + k) * P
                        nc.tensor.matmul(
                            pas[jg][:],
                            lhsT=gt[:, s0 : s0 + P],
                            rhs=m[:],
                            start=(c == 0 and k == 0),
                            stop=(c == NCH - 1 and k == KC - 1),
                        )
            return pas

        # ---------------- pass 1 ----------------
        for _rep in range(reps):
            for sg in range(NSG):
                fT = ftp.tile([P, SG * GRP], BF16, tag="fT")
                nc.sync.dma_start(
                    out=fT[:], in_=featT_t[:, sg * SG * GRP : (sg + 1) * SG * GRP]
                )
                pas = agg_supergroup(sg, feat_t, "g1", "m1", "pa1")
                for jg in range(SG):
                    g = sg * SG + jg
                    sl = slice(g * GRP, (g + 1) * GRP)
                    agg_s = misc.tile([P, GRP], BF16, tag="aggs")
                    nc.scalar.activation(
                        out=agg_s[:], in_=pas[jg][:],
                        func=mybir.ActivationFunctionType.Copy,
                    )
                    ph = ph_p.tile([D_HID, GRP], F32, tag="ph")
                    nc.tensor.matmul(
                        ph[:], lhsT=w0a_s[:], rhs=fT[:, jg * GRP : (jg + 1) * GRP],
                        start=True, stop=False,
                    )
                    nc.tensor.matmul(
                        ph[:], lhsT=w0b_s[:], rhs=agg_s[:], start=False, stop=True
                    )
                    nc.scalar.activation(
                        out=hT_s[:, sl], in_=ph[:],
                        func=mybir.ActivationFunctionType.Relu,
                    )
                    pt = pt_p.tile([P, P], BF16, tag="pt")
                    nc.tensor.transpose(
                        out=pt[:], in_=hT_s[:, sl], identity=ident_s[:]
                    )
                    hrow = misc.tile([P, P], BF16, tag="hrow")
                    nc.scalar.activation(
                        out=hrow[:], in_=pt[:],
                        func=mybir.ActivationFunctionType.Copy,
                    )
                    nc.sync.dma_start(out=h_piece[sl, :], in_=hrow[:])

            # ---------------- all-gather h ----------------
            if no_cc:
                nc.sync.dma_start(out=h_full[0:NPC, :], in_=h_piece[:])
            else:
                nc.gpsimd.collective_compute(
                    "AllGather",
                    mybir.AluOpType.bypass,
                    replica_groups=[list(range(C))],
                    ins=[h_piece[:]],
                    outs=[h_full[:]],
                )

            # ---------------- pass 2 ----------------
            for sg in range(NSG):
                pas = agg_supergroup(sg, h_full, "g2", "m2", "pa2")
                for jg in range(SG):
                    g = sg * SG + jg
                    sl = slice(g * GRP, (g + 1) * GRP)
                    agg_s = misc.tile([P, GRP], BF16, tag="agg2")
                    nc.scalar.activation(
                        out=agg_s[:], in_=pas[jg][:],
                        func=mybir.ActivationFunctionType.Copy,
                    )
                    po = pt_p.tile([D_OUT, GRP], F32, tag="po")
                    nc.tensor.matmul(
                        po[:], lhsT=w1a_s[:], rhs=hT_s[:, sl], start=True, stop=False
                    )
                    nc.tensor.matmul(
                        po[:], lhsT=w1b_s[:], rhs=agg_s[:], start=False, stop=True
                    )
                    o_s = misc.tile([D_OUT, GRP], F32, tag="os")
                    nc.scalar.activation(
                        out=o_s[:], in_=po[:],
                        func=mybir.ActivationFunctionType.Copy,
                    )
                    nc.sync.dma_start(out=outT_t[:, sl], in_=o_s[:])

    return nc


def prep_inputs(feat, edge_src, edge_dst, W0, W1, cfg):
    """Host-side index/layout prep. Returns per-core input maps."""
    N, D_IN = feat.shape
    N_PAD, D_HID, D_OUT = cfg["N_PAD"], cfg["D_HID"], cfg["D_OUT"]
    C, NPC, KC, NCH = cfg["C"], cfg["NPC"], cfg["KC"], cfg["NCH"]
    CH = N_PAD // NCH
    NG = NPC // GRP
    NSG = NG // SG
    CAP = KC * P
    E = edge_src.shape[0]
    bf16 = _np_bf16()

    indeg = np.bincount(edge_dst, minlength=N).astype(np.float32)
    ew = (1.0 / np.maximum(indeg, 1.0))[edge_dst].astype(np.float32)

    grp_of = edge_dst // GRP  # global group id
    chunk_of = edge_src // CH
    bucket = grp_of * NCH + chunk_of
    n_buckets = C * NG * NCH
    order = np.argsort(bucket, kind="stable")
    src_o = edge_src[order]
    dst_o = edge_dst[order]
    ew_o = ew[order]
    b_o = bucket[order]

    counts = np.bincount(b_o, minlength=n_buckets)
    if counts.max() > CAP:
        raise ValueError(f"bucket overflow: {counts.max()} > {CAP}")
    starts = np.zeros(n_buckets, dtype=np.int64)
    starts[1:] = np.cumsum(counts)[:-1]
    slot = np.arange(E, dtype=np.int64) - starts[b_o]

    idx16 = np.zeros((n_buckets, CAP), dtype=np.int16)
    dl = np.full((n_buckets, CAP), 2.0 * GRP, dtype=np.float32)
    wg = np.zeros((n_buckets, CAP), dtype=np.float32)
    idx16[b_o, slot] = (src_o % CH).astype(np.int16)
    dl[b_o, slot] = (dst_o % GRP).astype(np.float32)
    wg[b_o, slot] = ew_o

    # device layouts --------------------------------------------------
    # buckets: [C, NSG, SG(jg), NCH(c), KC, 128] from bucket id
    # (grp_global*NCH + c) where grp_global = core*NG + sg*SG + jg
    def to_dev(arr):
        return arr.reshape(C, NSG, SG, NCH, KC, P)

    idx6 = to_dev(idx16)
    dl6 = to_dev(dl)
    wg6 = to_dev(wg)

    # idx: per (core, sg, c) flat vector over (jg, k, pos), wrapped 16,
    # replicated to 128 partitions: [C, NSG, 128, NCH*GIDX]
    idxf = idx6.transpose(0, 1, 3, 2, 4, 5).reshape(C, NSG, NCH, SG * CAP)
    idxf = idxf.reshape(C, NSG, NCH, SG * CAP // 16, 16)
    idxf = idxf.transpose(0, 1, 4, 2, 3).reshape(C, NSG, 16, NCH * (SG * CAP // 16))
    idx_dev = np.ascontiguousarray(np.tile(idxf, (1, 1, 8, 1)))

    # dl/wg: [C, NSG, 128(pos), NCH*SG*KC] with col = c*SG*KC + jg*KC + k
    dl_dev = np.ascontiguousarray(
        dl6.transpose(0, 1, 5, 3, 2, 4).reshape(C, NSG, P, NCH * SG * KC)
    )
    wg_dev = np.ascontiguousarray(
        wg6.transpose(0, 1, 5, 3, 2, 4).reshape(C, NSG, P, NCH * SG * KC)
    )

    feat_pad = np.zeros((N_PAD, D_IN), dtype=np.float32)
    feat_pad[:N] = feat
    featbf = feat_pad.astype(bf16)
    featT = feat_pad.T.astype(bf16)  # [128, N_PAD]

    w0a = np.ascontiguousarray(W0[:, :D_IN].T).astype(bf16)
    w0b = np.ascontiguousarray(W0[:, D_IN:].T).astype(bf16)
    w1a = np.ascontiguousarray(W1[:, :D_HID].T).astype(bf16)
    w1b = np.ascontiguousarray(W1[:, D_HID:].T).astype(bf16)
    iota = np.tile(np.arange(GRP, dtype=np.float32), (P, 1)).astype(bf16)
    ident = np.eye(P, dtype=np.float32).astype(bf16)

    in_maps = []
    for c in range(C):
        in_maps.append(
            {
                "featbf": featbf,
                "featTbf": np.ascontiguousarray(featT[:, c * NPC : (c + 1) * NPC]),
                "idx16": idx_dev[c],
                "dl": dl_dev[c],
                "wg": wg_dev[c],
                "w0at": w0a,
                "w0bt": w0b,
                "w1at": w1a,
                "w1bt": w1b,
                "iota": iota,
                "ident": ident,
            }
        )
    return in_maps


_PROGRAM_CACHE = {}


def make_cfg(N, E, D_IN, D_HID, D_OUT, C=8):
    NPC = -(-N // (C * SG * GRP)) * (SG * GRP)  # per-core nodes, mult of 512
    N_PAD = C * NPC
    NCH = 4
    assert N_PAD % NCH == 0 and N_PAD // NCH <= 32768
    # capacity per (group, chunk) bucket: mean + 5 sigma, rounded to 128
    mean_b = E / (N / GRP) / NCH
    cap = mean_b + 5.0 * np.sqrt(mean_b) + 2
    KC = max(1, int(np.ceil(cap / P)))
    return {
        "N": N,
        "N_PAD": N_PAD,
        "D_IN": D_IN,
        "D_HID": D_HID,
        "D_OUT": D_OUT,
        "C": C,
        "NPC": NPC,
        "KC": KC,
        "NCH": NCH,
    }


def _build(feat, edge_src, edge_dst, W0, W1, C, reps):
    cfg = make_cfg(feat.shape[0], edge_src.shape[0], feat.shape[1], W0.shape[0], W1.shape[0], C)
    in_maps = None
    for _ in range(8):
        try:
            in_maps = prep_inputs(feat, edge_src, edge_dst, W0, W1, cfg)
            break
        except ValueError:
            cfg["KC"] += 1
    key = (tuple(sorted(cfg.items())), reps)
    if key not in _PROGRAM_CACHE:
        nc_new = build_program(cfg, reps=reps)
        nc_new.finalize()
        _PROGRAM_CACHE[key] = nc_new
    return _PROGRAM_CACHE[key], in_maps, cfg


def _run(feat, edge_src, edge_dst, W0, W1, C=8, trace=False):
    nc, in_maps, cfg = _build(feat, edge_src, edge_dst, W0, W1, C, 1)
    res = run_bass_kernel_spmd(nc, in_maps, core_ids=list(range(C)), trace=trace)
    pieces = [res.results[c]["outT"].T for c in range(C)]  # [NPC, D_OUT]
    out = np.concatenate(pieces, axis=0)[: cfg["N"]]
    return np.ascontiguousarray(out.astype(np.float32)), res


def bench(feat, edge_src, edge_dst, W0, W1, C=8, iters=10, reps=1):
    """Time device execution of the compiled program: inputs pre-staged on
    device, jit without donation, min over `iters` calls."""
    import time

    import jax
    from jax.sharding import Mesh, NamedSharding, PartitionSpec

    try:
        from jax.experimental.shard_map import shard_map
    except ImportError:
        from jax.shard_map import shard_map
    from concourse import bass2jax
    from concourse.bass2jax import _bass_exec_p

    feat = np.asarray(feat, dtype=np.float32)
    edge_src = np.asarray(edge_src, dtype=np.int32)
    edge_dst = np.asarray(edge_dst, dtype=np.int32)
    W0 = np.asarray(W0, dtype=np.float32)
    W1 = np.asarray(W1, dtype=np.float32)
    nc, in_maps, cfg = _build(feat, edge_src, edge_dst, W0, W1, C, reps)

    bass2jax.install_neuronx_cc_hook()
    import concourse.mybir as mb

    part_name = nc.partition_id_tensor.name if nc.partition_id_tensor else None
    in_names, out_names, out_avals, zero_outs = [], [], [], []
    for alloc in nc.m.functions[0].allocations:
        if not isinstance(alloc, mb.MemoryLocationSet):
            continue
        name = alloc.memorylocations[0].name
        if alloc.kind == "ExternalInput":
            if name != part_name:
                in_names.append(name)
        elif alloc.kind == "ExternalOutput":
            shape = tuple(alloc.tensor_shape)
            dtype = mb.dt.np(alloc.dtype)
            out_names.append(name)
            out_avals.append(jax.core.ShapedArray(shape, dtype))
            zero_outs.append(np.zeros(shape, dtype))
    n_params = len(in_names)
    all_in_names = in_names + out_names
    if part_name is not None:
        all_in_names.append(part_name)

    def _body(*args):
        operands = list(args)
        if part_name is not None:
            operands.append(bass2jax.partition_id_tensor())
        return tuple(
            _bass_exec_p.bind(
                *operands,
                out_avals=tuple(out_avals),
                in_names=tuple(all_in_names),
                out_names=tuple(out_names),
                lowering_input_output_aliases=(),
                sim_require_finite=True,
                sim_require_nnan=True,
                nc=nc,
            )
        )

    devices = jax.devices()[:C]
    mesh = Mesh(np.asarray(devices), ("core",))
    spec = PartitionSpec("core")
    n_args = n_params + len(out_names)
    fn = jax.jit(
        shard_map(
            _body,
            mesh=mesh,
            in_specs=(spec,) * n_args,
            out_specs=(spec,) * len(out_names),
            check_rep=False,
        )
    )
    concat_in = [
        np.concatenate([np.asarray(in_maps[c][nm]) for c in range(C)], axis=0)
        for nm in in_names
    ] + [np.zeros((C * z.shape[0], *z.shape[1:]), z.dtype) for z in zero_outs]
    sharding = NamedSharding(mesh, spec)
    dev_in = [jax.device_put(a, sharding) for a in concat_in]
    # warmup (compiles + first exec)
    r = fn(*dev_in)
    jax.block_until_ready(r)
    times = []
    for _ in range(iters):
        t0 = time.perf_counter()
        r = fn(*dev_in)
        jax.block_until_ready(r)
        times.append(time.perf_counter() - t0)
    return {
        "min_s": min(times),
        "median_s": sorted(times)[len(times) // 2],
        "all_s": times,
        "out": np.asarray(r[0]),
        "out_names": out_names,
        "cfg": cfg,
    }


def kernel(feat, edge_src, edge_dst, W0, W1):
    out, _ = _run(
        np.asarray(feat, dtype=np.float32),
        np.asarray(edge_src, dtype=np.int32),
        np.asarray(edge_dst, dtype=np.int32),
        np.asarray(W0, dtype=np.float32),
        np.asarray(W1, dtype=np.float32),
    )
    return out

